# revision 1
# baseline (speedup 1.0000x reference)
"""Trainium2 Bass kernel for int8 GEMM + fp32 bias (linear_a8_w8_bfp32_ofp32).

Computes out = (x_int8 @ weight_int8.T).astype(f32) + bias  for
x [8192, 4096] int8, weight [4096, 4096] int8, bias [4096] f32.

Strategy: column-parallel tensor parallelism over 8 NeuronCores — each core
gets all of x (replicated) and a 512-column slice of weight/bias, and
computes its [8192, 512] output slice.

The PE array has no int8 matmul mode (TRN2/cayman dropped UINT8), but
int8 values are exactly representable in bf16, bf16 x bf16 products
(<= 127*127) are exact, and PSUM accumulates in fp32 where every partial
sum of this data stays far below 2^24 — so a bf16 matmul reproduces the
int32-accumulated reference bit-exactly. fp8 can't beat this: an exact
int8 GEMM needs a >=3x nibble decomposition but DoubleRow only buys
~1.5-1.8x, so bf16 N=512 streaming (215.6 ns/MM) is the PE floor:
2048 MMs = 441.5 us/core.

Startup (measured constants): the SWDGE ring gives first data ~3.3us
after its first descriptor and sustains ~420GB/s of SBUF writes; HWDGE
(sync/scalar) queues are ~40-55GB/s and serial, so only bias/stores ride
them.  The PE warms up on garbage matmuls (memset on DVE so the ring
descriptors start immediately) sized to end exactly when the first real
k-tiles land, so the real MM stream starts at the warm 215.6ns rate with
no HAM re-throttle.  w k0-19 arrive raw int8 and are cast per-k-tile in
ring-arrival order (DVE: evens + k13/15/17/19; scalar: odds k1-11); w
k20-31 arrive as SWDGE casting DMAs (bf16 direct) because engine cast
throughput runs out exactly there.  x rides casting DMAs throughout.
The last m-tile is split into two 256-wide PSUM chains so the first
half's bias-add + store hide behind the second half's matmuls.
"""

import numpy as np

import concourse.mybir as mybir
import concourse.tile as tile
from concourse import bacc
from concourse.bass_utils import run_bass_kernel_spmd

P = 128
N_CORES = 8

# Set by a test harness to capture timing/trace info; harmless defaults.
TRACE = False
TRACE_KWARGS = {}
LAST_RESULT = None


def build_program(MT, KT, NLOC, x_bufs=4, o_bufs=3, psum_bufs=4, warmup_mms=19):
    """Bass/Tile program for one core: out[MT*128, NLOC] = xT.T @ wT + bias.

    DRAM layouts (host pre-arranged, all contiguous per SBUF partition):
      x_tiles   [MT, P, KT, P]  int8   x_tiles[mt, ki, kt, mi] = x[mt*P+mi, kt*P+ki]
      w_tiles   [P, KT, NLOC]   int8   w_tiles[ki, kt, n] = weight[n, kt*P+ki]
      bias_bcast[P, NLOC]       f32    bias replicated across partitions
      out_tiles [MT, P, NLOC]   f32    out_tiles[mt, mi, n] = out[mt*P+mi, n]
    """
    nc = bacc.Bacc()
    x_d = nc.declare_dram_parameter(
        "x_tiles", [MT, P, KT, P], mybir.dt.int8, isOutput=False
    )
    w_d = nc.declare_dram_parameter(
        "w_tiles", [P, KT, NLOC], mybir.dt.int8, isOutput=False
    )
    b_d = nc.declare_dram_parameter(
        "bias_bcast", [P, NLOC], mybir.dt.float32, isOutput=False
    )
    o_d = nc.declare_dram_parameter(
        "out_tiles", [MT, P, NLOC], mybir.dt.float32, isOutput=True
    )

    KC = 4  # k-tiles per w chunk
    NH = NLOC // 2
    N_RAW = 5  # w chunks 0-4 (k0-19) raw + engine cast; 5-7 casting DMA

    with tile.TileContext(nc) as tc:
        with (
            tc.tile_pool(name="wkpool", bufs=1) as wkpool,
            tc.tile_pool(name="wqpool", bufs=1) as wqpool,
            tc.tile_pool(name="cpool", bufs=1) as cpool,
            tc.tile_pool(name="x0pool", bufs=1) as x0pool,
            tc.tile_pool(name="xpool", bufs=x_bufs) as xpool,
            tc.tile_pool(name="opool", bufs=o_bufs) as opool,
            tc.tile_pool(name="otail", bufs=2) as otail,
            tc.tile_pool(name="psum", bufs=psum_bufs, space="PSUM") as psum_pool,
            tc.tile_pool(name="pst", bufs=1, space="PSUM") as pst_pool,
            tc.tile_pool(name="warm", bufs=1) as warm_pool,
            tc.tile_pool(name="warm_ps", bufs=1, space="PSUM") as warm_psum,
        ):
            # PE warmup: garbage matmuls keep the PE busy from ~7us so the
            # HAM un-throttles to 2.4GHz and stays warm; the count is sized
            # so the warmup drains right as the first real k-tiles land.
            # memset on DVE so the gpsimd queue starts descriptor-gen
            # immediately.
            if warmup_mms:
                wu = warm_pool.tile([P, NLOC], mybir.dt.bfloat16)
                nc.vector.memset(wu[:], 0.0)
                wu_ps = warm_psum.tile([P, NLOC], mybir.dt.float32)
                for i in range(warmup_mms):
                    nc.tensor.matmul(
                        wu_ps[:],
                        wu[:, :P],
                        wu[:],
                        start=(i == 0),
                        stop=(i == warmup_mms - 1),
                    )

            # gpsimd SWDGE ring, strict FIFO, baseline-proven order:
            # x0/w chunks interleaved, then w4 raw, w5-7 as casting DMAs
            # (bf16 direct), then x1 in two halves, then the m-loop x
            # tiles.  All x DMAs cast int8->bf16 in the DMA.
            x0_sb = []
            wq_raw = []
            wb_cast = {}

            def emit_w_raw(j):
                wq_t = wqpool.tile(
                    [P, KC, NLOC], mybir.dt.int8, tag=f"wq{j}", name=f"wq{j}"
                )
                nc.gpsimd.dma_start(out=wq_t[:], in_=w_d[:, j * KC : (j + 1) * KC, :])
                wq_raw.append(wq_t)

            XC = 4  # x0 split into 4 chunks of KT//XC k-tiles
            KXC = KT // XC
            for j in range(XC):
                x_c = x0pool.tile(
                    [P, KXC, P], mybir.dt.bfloat16, tag=f"x0c{j}", name=f"x0c{j}"
                )
                nc.gpsimd.dma_start(
                    out=x_c[:], in_=x_d[0, :, j * KXC : (j + 1) * KXC, :]
                )
                x0_sb.append(x_c)
                emit_w_raw(j)
            emit_w_raw(4)
            for j in range(N_RAW, KT // KC):
                wb = wkpool.tile(
                    [P, KC, NLOC], mybir.dt.bfloat16, tag=f"wb{j}", name=f"wb{j}"
                )
                nc.gpsimd.dma_start(out=wb[:], in_=w_d[:, j * KC : (j + 1) * KC, :])
                wb_cast[j] = wb
            x1lo = x0pool.tile([P, 16, P], mybir.dt.bfloat16, tag="x1lo")
            nc.gpsimd.dma_start(out=x1lo[:], in_=x_d[1, :, 0:16, :])
            x1hi = x0pool.tile([P, 16, P], mybir.dt.bfloat16, tag="x1hi")
            nc.gpsimd.dma_start(out=x1hi[:], in_=x_d[1, :, 16:32, :])

            # bias on the (otherwise idle until stores) sync HWDGE queue.
            b_sb = cpool.tile([P, NLOC], mybir.dt.float32)
            nc.sync.dma_start(out=b_sb[:], in_=b_d[:])

            # --- w casts for k0-19, per k-tile, in ring-arrival order ----
            wk = {}
            for k in range(N_RAW * KC):
                wk[k] = wkpool.tile(
                    [P, NLOC], mybir.dt.bfloat16, tag=f"wk{k}", name=f"wk{k}"
                )

            def w_stage_slice(k):
                return wq_raw[k // KC][:, k % KC, :]

            # DVE: evens k0-18 plus odds k13,k15,k17,k19 (scalar is ~0.71us
            # per tile and can only cover the first few odds in time).
            dve_ks = []
            for j in range(N_RAW):
                for k in range(j * KC, (j + 1) * KC):
                    if k % 2 == 0 or k >= 13:
                        dve_ks.append(k)
            for k in dve_ks:
                nc.vector.tensor_copy(wk[k][:], w_stage_slice(k))
            for k in (1, 3, 5, 7, 9, 11):
                nc.scalar.copy(wk[k][:], w_stage_slice(k))

            def w_slice(kt):
                if kt < N_RAW * KC:
                    return wk[kt][:]
                return wb_cast[kt // KC][:, kt % KC, :]

            # --- main m-tile loop -----------------------------------------
            def x_slice(mt, x_sb, kt):
                if mt == 0:
                    return x0_sb[kt // KXC][:, kt % KXC, :]
                if mt == 1:
                    if kt < 16:
                        return x1lo[:, kt, :]
                    return x1hi[:, kt - 16, :]
                return x_sb[:, kt, :]

            for mt in range(MT):
                if mt <= 1:
                    x_sb = None
                else:
                    x_sb = xpool.tile([P, KT, P], mybir.dt.bfloat16)
                    nc.gpsimd.dma_start(out=x_sb[:], in_=x_d[mt])
                if mt < MT - 1:
                    ps = psum_pool.tile([P, NLOC], mybir.dt.float32)
                    for kt in range(KT):
                        nc.tensor.matmul(
                            ps[:],
                            x_slice(mt, x_sb, kt),
                            w_slice(kt),
                            start=(kt == 0),
                            stop=(kt == KT - 1),
                        )
                    o_sb = opool.tile([P, NLOC], mybir.dt.float32)
                    nc.vector.tensor_add(o_sb[:], ps[:], b_sb[:])
                    nc.sync.dma_start(out=o_d[mt], in_=o_sb[:])
                else:
                    # last m-tile: progressively narrower chains (256, 128,
                    # 128 cols) so each epilogue hides behind the next
                    # chain's matmuls and the final exposed tail is only a
                    # quarter-width bias-add + store.
                    NQ = NLOC // 4
                    spans = [(0, NH), (NH, NH + NQ), (NH + NQ, NLOC)]
                    engs = [nc.sync, nc.scalar, nc.sync]
                    for h, (n0, n1) in enumerate(spans):
                        ph = pst_pool.tile(
                            [P, n1 - n0], mybir.dt.float32,
                            tag=f"pst{h}", name=f"pst{h}",
                        )
                        for kt in range(KT):
                            w_ap = w_slice(kt)
                            nc.tensor.matmul(
                                ph[:],
                                x_slice(mt, x_sb, kt),
                                w_ap[:, n0:n1],
                                start=(kt == 0),
                                stop=(kt == KT - 1),
                            )
                        o_h = otail.tile(
                            [P, n1 - n0], mybir.dt.float32,
                            tag=f"ot{h}", name=f"ot{h}",
                        )
                        nc.vector.tensor_add(o_h[:], ph[:], b_sb[:, n0:n1])
                        engs[h].dma_start(out=o_d[mt, :, n0:n1], in_=o_h[:])
    nc.compile()
    return nc


def run(x, weight, fake_bias):
    global LAST_RESULT
    M, K = x.shape
    N = weight.shape[0]
    assert M % P == 0 and K % P == 0 and N % (N_CORES * P) == 0
    MT, KT, NLOC = M // P, K // P, N // N_CORES

    xb = np.asarray(x).astype(np.int8)
    x_tiles = np.ascontiguousarray(xb.reshape(MT, P, KT, P).transpose(0, 3, 2, 1))
    wb = np.asarray(weight).astype(np.int8)
    bias = np.asarray(fake_bias).astype(np.float32)

    in_maps = []
    for c in range(N_CORES):
        w_loc = wb[c * NLOC : (c + 1) * NLOC, :]  # [NLOC, K]
        w_tiles = np.ascontiguousarray(
            w_loc.T.reshape(KT, P, NLOC).transpose(1, 0, 2)
        )
        b_loc = np.ascontiguousarray(
            np.broadcast_to(bias[None, c * NLOC : (c + 1) * NLOC], (P, NLOC))
        )
        in_maps.append(
            {"x_tiles": x_tiles, "w_tiles": w_tiles, "bias_bcast": b_loc}
        )

    nc = build_program(MT, KT, NLOC)
    res = run_bass_kernel_spmd(
        nc, in_maps, list(range(N_CORES)), trace=TRACE, **TRACE_KWARGS
    )
    LAST_RESULT = res

    outs = [r["out_tiles"].reshape(M, NLOC) for r in res.results]
    return np.concatenate(outs, axis=1).astype(np.float32)


def kernel(x, weight, fake_bias):
    return run(x, weight, fake_bias)



# revision 8
# speedup vs baseline: 1.0022x; 1.0022x over previous
"""Trainium2 Bass kernel for int8 GEMM + fp32 bias (linear_a8_w8_bfp32_ofp32).

Computes out = (x_int8 @ weight_int8.T).astype(f32) + bias  for
x [8192, 4096] int8, weight [4096, 4096] int8, bias [4096] f32.

Strategy: column-parallel tensor parallelism over 8 NeuronCores — each core
gets all of x (replicated) and a 512-column slice of weight/bias, and
computes its [8192, 512] output slice.

The PE array has no int8 matmul mode (TRN2/cayman dropped UINT8), but
int8 values are exactly representable in bf16, bf16 x bf16 products
(<= 127*127) are exact, and PSUM accumulates in fp32 where every partial
sum of this data stays far below 2^24 — so a bf16 matmul reproduces the
int32-accumulated reference bit-exactly. fp8 can't beat this: an exact
int8 GEMM needs a >=3x nibble decomposition but DoubleRow only buys 2x,
so bf16 N=512 streaming (215.8 ns/MM) is the PE floor.

Measured trace model (ntff):
- exec_time = last_useful - first_useful; first_useful ~6.1us (framework
  preamble excluded), last_useful = end of walrus's fixed epilogue that
  zeroes all 255 semaphores one EVENT_SEMAPHORE each, split across the 5
  engines (~6.8us, Tensor slowest at ~115ns/op). Fixed cost; every ns the
  real work finishes earlier shifts it 1:1.
- SWDGE ring: first chunk data ~4.7us after first descriptor, then
  ~0.64us per 256KB chunk; descriptor-gen ~0.67us each on GpSimd.
- HAM: first ~10 MMs run at half clock (427ns), unthrottles after ~4us
  of PE activity; steady MM issue = 215.8ns (N=512).
- DVE int8->bf16 cast of [128,512]: 427ns; scalar ACTIVATE copy: 715ns.

Ramp design: m-tiles 0 and 1 run as two interleaved PSUM chains
(A-k0,B-k0,A-k1,...), halving k-tile consumption to 432ns/k so the ring
(w raw chunks + x chunks round-robin) and the DVE(+scalar every 4th)
casts keep pace with ~25% margin — no stalls. Warmup is 14 garbage MMs
(10 cold-clock + 4 warm) sized to drain right as w-k0's cast lands.
All w arrives raw int8 (2MB vs 4MB bf16 on the ring) and is cast by
DVE (k%4!=3) / scalar (k%4==3) in ring-arrival order.

Tail: last m-tile splits into 256/128/128-wide chains so epilogues hide
behind the next chain's MMs; chain stores go to the scalar HWDGE queue
(idle after the casts) and sync, with the final 128-wide store split
32KB+32KB across both queues in parallel — final DMA drains ~0.7us
after the last MM, pulling the whole teardown forward.
"""

import numpy as np

import concourse.mybir as mybir
import concourse.tile as tile
from concourse import bacc
from concourse.bass_utils import run_bass_kernel_spmd

P = 128
N_CORES = 8

# Set by a test harness to capture timing/trace info; harmless defaults.
TRACE = False
TRACE_KWARGS = {}
LAST_RESULT = None


def build_program(MT, KT, NLOC, x_bufs=4, o_bufs=3, psum_bufs=3, warmup_mms=14):
    """Bass/Tile program for one core: out[MT*128, NLOC] = xT.T @ wT + bias.

    DRAM layouts (host pre-arranged, all contiguous per SBUF partition):
      x_tiles   [MT, P, KT, P]  int8   x_tiles[mt, ki, kt, mi] = x[mt*P+mi, kt*P+ki]
      w_tiles   [P, KT, NLOC]   int8   w_tiles[ki, kt, n] = weight[n, kt*P+ki]
      bias_bcast[P, NLOC]       f32    bias replicated across partitions
      out_tiles [MT, P, NLOC]   f32    out_tiles[mt, mi, n] = out[mt*P+mi, n]
    """
    nc = bacc.Bacc()
    x_d = nc.declare_dram_parameter(
        "x_tiles", [MT, P, KT, P], mybir.dt.int8, isOutput=False
    )
    w_d = nc.declare_dram_parameter(
        "w_tiles", [P, KT, NLOC], mybir.dt.int8, isOutput=False
    )
    b_d = nc.declare_dram_parameter(
        "bias_bcast", [P, NLOC], mybir.dt.float32, isOutput=False
    )
    o_d = nc.declare_dram_parameter(
        "out_tiles", [MT, P, NLOC], mybir.dt.float32, isOutput=True
    )

    NH = NLOC // 2

    with tile.TileContext(nc) as tc:
        with (
            tc.tile_pool(name="wqpool", bufs=1) as wqpool,
            tc.tile_pool(name="wkpool", bufs=1) as wkpool,
            tc.tile_pool(name="cpool", bufs=1) as cpool,
            tc.tile_pool(name="x01pool", bufs=1) as x01pool,
            tc.tile_pool(name="xpool", bufs=x_bufs) as xpool,
            tc.tile_pool(name="opool", bufs=o_bufs) as opool,
            tc.tile_pool(name="otail", bufs=2) as otail,
            tc.tile_pool(name="psum", bufs=psum_bufs, space="PSUM") as psum_pool,
            tc.tile_pool(name="psab", bufs=1, space="PSUM") as psab_pool,
            tc.tile_pool(name="pst", bufs=1, space="PSUM") as pst_pool,
            tc.tile_pool(name="warm", bufs=1) as warm_pool,
        ):
            # PE warmup: garbage matmuls un-throttle the HAM (~10 MMs at
            # half clock, then full speed) and end right as the first real
            # k-tile's cast completes.  memset on DVE so the gpsimd queue
            # starts ring descriptor-gen immediately.  Accumulates into
            # chain A's PSUM bank (never read; chain A's start=True MM
            # resets it).
            ps_a = psab_pool.tile([P, NLOC], mybir.dt.float32, tag="psA", name="psA")
            ps_b = psab_pool.tile([P, NLOC], mybir.dt.float32, tag="psB", name="psB")
            if warmup_mms:
                wu = warm_pool.tile([P, NLOC], mybir.dt.bfloat16)
                nc.vector.memset(wu[:], 0.0)
                for i in range(warmup_mms):
                    nc.tensor.matmul(
                        ps_a[:],
                        wu[:, :P],
                        wu[:],
                        start=(i == 0),
                        stop=(i == warmup_mms - 1),
                    )

            # gpsimd SWDGE ring, strict FIFO.  Startup chunks round-robin
            # x0/w/x1 so m0+m1's dual-chain consumption (432ns/k-tile)
            # never outruns arrivals; later chunks are merged (fewer
            # descriptor-gen slots, same bytes).  All x DMAs cast
            # int8->bf16 in the DMA; w stays raw int8 (half the ring
            # bytes) and is cast by DVE/scalar below.
            x0_sb = {}
            x1_sb = {}
            wq_raw = {}

            def emit_x(m, tag, k0, k1):
                t = x01pool.tile(
                    [P, k1 - k0, P], mybir.dt.bfloat16,
                    tag=f"x{m}c{tag}", name=f"x{m}c{tag}",
                )
                nc.gpsimd.dma_start(out=t[:], in_=x_d[m, :, k0:k1, :])
                (x0_sb if m == 0 else x1_sb)[k0] = t

            def emit_w(tag, k0, k1):
                t = wqpool.tile(
                    [P, k1 - k0, NLOC], mybir.dt.int8,
                    tag=f"wq{tag}", name=f"wq{tag}",
                )
                nc.gpsimd.dma_start(out=t[:], in_=w_d[:, k0:k1, :])
                wq_raw[k0] = (t, k1)

            emit_x(0, 0, 0, 8)
            emit_w(0, 0, 4)
            emit_x(1, 0, 0, 8)
            emit_w(1, 4, 8)
            emit_x(0, 1, 8, 16)
            emit_w(2, 8, 12)
            emit_x(1, 1, 8, 16)
            emit_w(3, 12, 16)
            emit_x(0, 2, 16, 32)
            emit_w(45, 16, 24)
            emit_x(1, 2, 16, 32)
            emit_w(67, 24, 32)

            # bias on the (otherwise idle until stores) sync HWDGE queue.
            b_sb = cpool.tile([P, NLOC], mybir.dt.float32)
            nc.sync.dma_start(out=b_sb[:], in_=b_d[:])

            # --- w casts, in ring-arrival (= k) order; scalar takes every
            # 4th tile so DVE (427ns/tile vs 432ns/k dual-chain pace plus
            # chunk-arrival slack) always has margin.
            def w_stage_slice(k):
                for k0, (t, k1) in wq_raw.items():
                    if k0 <= k < k1:
                        return t[:, k - k0, :]
                raise KeyError(k)

            wk = {}
            for k in range(KT):
                wk[k] = wkpool.tile(
                    [P, NLOC], mybir.dt.bfloat16, tag=f"wk{k}", name=f"wk{k}"
                )
                if k % 4 == 3:
                    nc.scalar.copy(wk[k][:], w_stage_slice(k))
                else:
                    nc.vector.tensor_copy(wk[k][:], w_stage_slice(k))

            def x01_slice(sb, kt):
                if kt < 8:
                    return sb[0][:, kt, :]
                if kt < 16:
                    return sb[8][:, kt - 8, :]
                return sb[16][:, kt - 16, :]

            # --- m-tiles 0+1: interleaved dual PSUM chains ---------------
            for kt in range(KT):
                nc.tensor.matmul(
                    ps_a[:], x01_slice(x0_sb, kt), wk[kt][:],
                    start=(kt == 0), stop=(kt == KT - 1),
                )
                nc.tensor.matmul(
                    ps_b[:], x01_slice(x1_sb, kt), wk[kt][:],
                    start=(kt == 0), stop=(kt == KT - 1),
                )
            for mt, ps in ((0, ps_a), (1, ps_b)):
                o_sb = opool.tile([P, NLOC], mybir.dt.float32)
                nc.vector.tensor_add(o_sb[:], ps[:], b_sb[:])
                nc.sync.dma_start(out=o_d[mt], in_=o_sb[:])

            # --- main m-tile loop ----------------------------------------
            for mt in range(2, MT):
                x_sb = xpool.tile([P, KT, P], mybir.dt.bfloat16)
                nc.gpsimd.dma_start(out=x_sb[:], in_=x_d[mt])
                if mt < MT - 1:
                    ps = psum_pool.tile([P, NLOC], mybir.dt.float32)
                    for kt in range(KT):
                        nc.tensor.matmul(
                            ps[:],
                            x_sb[:, kt, :],
                            wk[kt][:],
                            start=(kt == 0),
                            stop=(kt == KT - 1),
                        )
                    o_sb = opool.tile([P, NLOC], mybir.dt.float32)
                    nc.vector.tensor_add(o_sb[:], ps[:], b_sb[:])
                    nc.sync.dma_start(out=o_d[mt], in_=o_sb[:])
                else:
                    # last m-tile: progressively narrower chains so each
                    # epilogue hides behind the next chain's matmuls.
                    # Stores ride the scalar HWDGE queue (idle since the
                    # casts) except the final chain, whose 64KB store is
                    # split 32+32KB across sync and scalar so the last
                    # drain is ~0.3us after its bias-add.
                    NQ = NLOC // 4
                    spans = [(0, NH), (NH, NH + NQ), (NH + NQ, NLOC)]
                    for h, (n0, n1) in enumerate(spans):
                        ph = pst_pool.tile(
                            [P, n1 - n0], mybir.dt.float32,
                            tag=f"pst{h}", name=f"pst{h}",
                        )
                        for kt in range(KT):
                            nc.tensor.matmul(
                                ph[:],
                                x_sb[:, kt, :],
                                wk[kt][:, n0:n1],
                                start=(kt == 0),
                                stop=(kt == KT - 1),
                            )
                        o_h = otail.tile(
                            [P, n1 - n0], mybir.dt.float32,
                            tag=f"ot{h}", name=f"ot{h}",
                        )
                        nc.vector.tensor_add(o_h[:], ph[:], b_sb[:, n0:n1])
                        if h == 0:
                            nc.scalar.dma_start(out=o_d[mt, :, n0:n1], in_=o_h[:])
                        elif h == 1:
                            nc.sync.dma_start(out=o_d[mt, :, n0:n1], in_=o_h[:])
                        else:
                            nh = (n1 - n0) // 2
                            nc.sync.dma_start(
                                out=o_d[mt, :, n0:n0 + nh], in_=o_h[:, :nh]
                            )
                            nc.scalar.dma_start(
                                out=o_d[mt, :, n0 + nh:n1], in_=o_h[:, nh:]
                            )
    nc.compile()
    return nc


def run(x, weight, fake_bias):
    global LAST_RESULT
    M, K = x.shape
    N = weight.shape[0]
    assert M % P == 0 and K % P == 0 and N % (N_CORES * P) == 0
    MT, KT, NLOC = M // P, K // P, N // N_CORES

    xb = np.asarray(x).astype(np.int8)
    x_tiles = np.ascontiguousarray(xb.reshape(MT, P, KT, P).transpose(0, 3, 2, 1))
    wb = np.asarray(weight).astype(np.int8)
    bias = np.asarray(fake_bias).astype(np.float32)

    in_maps = []
    for c in range(N_CORES):
        w_loc = wb[c * NLOC : (c + 1) * NLOC, :]  # [NLOC, K]
        w_tiles = np.ascontiguousarray(
            w_loc.T.reshape(KT, P, NLOC).transpose(1, 0, 2)
        )
        b_loc = np.ascontiguousarray(
            np.broadcast_to(bias[None, c * NLOC : (c + 1) * NLOC], (P, NLOC))
        )
        in_maps.append(
            {"x_tiles": x_tiles, "w_tiles": w_tiles, "bias_bcast": b_loc}
        )

    nc = build_program(MT, KT, NLOC)
    res = run_bass_kernel_spmd(
        nc, in_maps, list(range(N_CORES)), trace=TRACE, **TRACE_KWARGS
    )
    LAST_RESULT = res

    outs = [r["out_tiles"].reshape(M, NLOC) for r in res.results]
    return np.concatenate(outs, axis=1).astype(np.float32)


def kernel(x, weight, fake_bias):
    return run(x, weight, fake_bias)
